# revision 1
# baseline (speedup 1.0000x reference)
"""Trainium2 Bass kernel for EnhancedGNN (3x GCNConv + mean-pool + FC), v2.

Self-contained: host-side sharding/layout prep + SPMD Bass/Tile program on 8
NeuronCores. See bottom for the `kernel(**inputs)` entry point.

v2 design (vs v1):
  - GCN norm (deg^-1/2 scaling) folded into per-edge weights on host; the
    device sees plain bf16 S-matrix weights.
  - G tensors (scaled features gathered per edge) stored bf16 [TOTAL, 128];
    F=64 layers write [g|g] duplicated rows via duplicated weight matrices so
    every gather element is a full 256B of initialized data.
  - Aggregation matmuls run transposed for layers 1-2 (psum = G_chunk^T @ S)
    so the next layer's input appears feature-major, removing per-tile
    transposes; bias + relu fold into one ACT op with per-partition bias.
  - G2/G3 are produced in the aggregation epilogues (no standalone gemm for
    layers 2/3) and AllGathered; layer 1's G1 is computed replicated from the
    bf16 x input (no collective).
  - One-hot S matrices built per bucket (not per chunk) with stride-0
    broadcast tensor_tensor ops; per-edge weights multiplied into the
    gathered rows, also per bucket.
  - Variable chunk counts per (group, block, tile) bucket, baked at compile.
"""

import os
import sys

import numpy as np

for _p in ("/opt/trn_rl_repo", "/root/.axon_site", "/root/.axon_site/_ro/pypackages"):
    if os.path.isdir(_p) and _p not in sys.path:
        sys.path.append(_p)

import ml_dtypes

BF16 = ml_dtypes.bfloat16
P = 128


def cdiv(a, b):
    return -(-a // b)


class Cfg:
    def __init__(self, n_nodes, n_edges, nc, tiles_pc, grp, nblk, n_graphs):
        self.N = n_nodes
        self.E = n_edges
        self.NC = nc
        self.T = tiles_pc
        self.GRP = grp
        self.NBLK = nblk
        self.G = n_graphs
        self.NPC = self.T * P
        self.TOTAL = self.NC * self.NPC
        self.BLK = self.TOTAL // self.NBLK
        assert self.T % self.GRP == 0
        assert self.TOTAL % self.NBLK == 0
        assert self.BLK <= 32768
        assert self.N % self.NC == 0
        assert self.N // self.NC <= self.NPC
        self.F = (64, 64, 128, 64)  # F0(in), F1, F2, F3
        # filled by host_prep:
        self.CHTS = None      # {(g,b,j): n_chunks}
        self.CBASE = None     # {(g,b,j): first chunk col}
        self.NCHUNK = None    # total chunks
        self.CHT_MAX = None
        self.HAS_B3 = False


FULL_CFG = dict(n_nodes=100000, n_edges=3200000, nc=8, tiles_pc=98, grp=7,
                nblk=4, n_graphs=64)


# --------------------------------------------------------------------------
# Host-side prep: node assignment, edge bucketing, layout arrays.
# --------------------------------------------------------------------------

def host_prep(x, src, dst, edge_weight, batch, W1, b1, W2, b2, W3, b3, Wfc,
              bfc, cfg: Cfg):
    N, E, NC, T = cfg.N, cfg.E, cfg.NC, cfg.T
    NPC, TOTAL, NBLK, BLK, GRP = cfg.NPC, cfg.TOTAL, cfg.NBLK, cfg.BLK, cfg.GRP
    NGRP = T // GRP
    GR = GRP * P
    F0 = cfg.F[0]
    x = np.ascontiguousarray(np.asarray(x, np.float32))
    src = np.asarray(src).astype(np.int64)
    dst = np.asarray(dst).astype(np.int64)
    ew = np.asarray(edge_weight, np.float32)
    batch = np.asarray(batch).astype(np.int64)

    # ---- node -> (core, tile, p) assignment, degree balanced ----
    degc = np.bincount(dst, minlength=N)
    order = np.argsort(-degc, kind="stable")
    ranks = np.arange(N)
    core_of = np.empty(N, np.int64)
    rank_in_core = np.empty(N, np.int64)
    core_of[order] = ranks % NC
    rank_in_core[order] = ranks // NC
    row = rank_in_core // T
    col = rank_in_core % T
    tile = np.where(row % 2 == 0, col, T - 1 - col)
    p_in_tile = row
    assert p_in_tile.max() < P
    # G-row id: within (core, group): p * GRP + j so group writes are
    # per-partition contiguous
    g_i = tile // GRP
    j_i = tile % GRP
    grow = core_of * NPC + g_i * GR + p_in_tile * GRP + j_i

    # ---- self loops + GCN norm folded into edge weights (host) ----
    loop = np.arange(N, dtype=np.int64)
    src_f = np.concatenate([src, loop])
    dst_f = np.concatenate([dst, loop])
    ew_f = np.concatenate([ew, np.ones(N, np.float32)])
    deg = np.bincount(dst_f, weights=ew_f.astype(np.float64),
                      minlength=N).astype(np.float32)
    dinv = np.where(deg > 0, 1.0 / np.sqrt(deg), 0.0).astype(np.float32)
    norm = dinv[src_f] * ew_f * dinv[dst_f]

    # ---- edge bucketing by (dst core, group g, src block b, tile j) ----
    e_core = core_of[dst_f]
    e_g = g_i[dst_f]
    e_j = j_i[dst_f]
    e_p = p_in_tile[dst_f]
    e_grow = grow[src_f]
    e_B = e_grow // BLK
    e_lidx = (e_grow % BLK).astype(np.int64)
    key = ((e_core * NGRP + e_g) * NBLK + e_B) * GRP + e_j
    si = np.argsort(key, kind="stable")
    key_s = key[si]
    nbuck = NC * NGRP * NBLK * GRP
    bc = np.bincount(key_s, minlength=nbuck).reshape(NC, NGRP * NBLK * GRP)
    # per-core chunk counts must be IDENTICAL across cores for SPMD (one
    # program): use per-bucket max over cores.
    chts_flat = cdiv(bc, P).max(axis=0)  # [NGRP*NBLK*GRP]
    cbase_flat = np.zeros(chts_flat.size + 1, np.int64)
    np.cumsum(chts_flat, out=cbase_flat[1:])
    nchunk = int(cbase_flat[-1])
    cfg.NCHUNK = nchunk
    cfg.CHT_MAX = int(chts_flat.max())
    CHTS = {}
    CBASE = {}
    for g in range(NGRP):
        for b in range(NBLK):
            for j in range(GRP):
                f = (g * NBLK + b) * GRP + j
                CHTS[(g, b, j)] = int(chts_flat[f])
                CBASE[(g, b, j)] = int(cbase_flat[f])
    cfg.CHTS = CHTS
    cfg.CBASE = CBASE

    # ---- slot assignment within buckets ----
    starts = np.zeros(nbuck + 1, np.int64)
    np.cumsum(bc.reshape(-1), out=starts[1:])
    slot = np.arange(E + N) - starts[key_s]
    core_b = key_s // (NGRP * NBLK * GRP)
    buck = key_s % (NGRP * NBLK * GRP)

    # idx / dstf / wf arrays, chunk-column layout
    idx_arr = np.zeros((NC, nchunk * P), np.int16)
    dstf = np.full((NC, P, nchunk), -1.0, np.float32)
    wff = np.zeros((NC, P, nchunk), np.float32)
    ccol = cbase_flat[buck] + slot // P     # global chunk column
    pp = slot % P
    idx_arr[core_b, ccol * P + pp] = e_lidx[si].astype(np.int16)
    dstf[core_b, pp, ccol] = e_p[si].astype(np.float32)
    wff[core_b, pp, ccol] = norm[si]

    # 16-wrap the indices per chunk: slot e of chunk c -> [e%16, c*8 + e//16],
    # replicated x8 along partitions.
    idx16 = idx_arr.reshape(NC, nchunk, 8, 16).transpose(0, 3, 1, 2)
    idx16 = idx16.reshape(NC, 16, nchunk * 8)
    idx16 = np.ascontiguousarray(
        np.broadcast_to(idx16[:, None, :, :], (NC, 8, 16, nchunk * 8))
        .reshape(NC, P, nchunk * 8))

    # ---- batch one-hot source values (per tile col), pad -> -1 ----
    batchf = np.full((NC, P, T), -1.0, np.float32)
    batchf[core_of, p_in_tile, tile] = batch.astype(np.float32)

    # ---- features transposed into grow order, bf16 ----
    xT = np.zeros((F0, TOTAL), np.float32)
    xT[:, grow] = x.T

    # ---- constants ----
    iota = np.tile(np.arange(P, dtype=np.float32)[None, :], (P, 1))
    ident64 = np.eye(64, dtype=np.float32)
    cnts = np.maximum(np.bincount(batch, minlength=cfg.G).astype(np.float32),
                      1.0)
    cinv = (1.0 / cnts).reshape(cfg.G, 1)

    W1 = np.asarray(W1, np.float32)
    W2 = np.asarray(W2, np.float32)
    W3 = np.asarray(W3, np.float32)
    b1 = np.asarray(b1, np.float32).reshape(-1)
    b2 = np.asarray(b2, np.float32).reshape(-1)
    b3 = np.asarray(b3, np.float32).reshape(-1)
    cfg.HAS_B3 = bool(np.any(b3 != 0.0))
    assert not cfg.HAS_B3, "nonzero b3 not supported in this kernel version"
    W1dup = np.concatenate([W1, W1], axis=1)          # [64, 128]
    W3dup = np.concatenate([W3, W3], axis=1)          # [128, 128]

    per_core = []
    for c in range(NC):
        m = {
            "xT": xT.astype(BF16),
            "idx16": np.ascontiguousarray(idx16[c]),
            "dstf": np.ascontiguousarray(dstf[c]).astype(BF16),
            "wf": np.ascontiguousarray(wff[c]).astype(BF16),
            "dstf32": np.ascontiguousarray(dstf[c]),
            "wf32": np.ascontiguousarray(wff[c]),
            "batchf": np.ascontiguousarray(batchf[c]),
            "iota": iota.astype(BF16),
            "ident64": ident64,
            "cinv": cinv,
            "W1dup": W1dup.astype(BF16),
            "W2": W2.astype(BF16),
            "W3dup": W3dup.astype(BF16),
            "Wfc": np.asarray(Wfc, np.float32).reshape(cfg.F[3], 1),
            "b1c": b1.reshape(cfg.F[1], 1),
            "b2c": b2.reshape(cfg.F[2], 1),
            "b3r": b3.reshape(1, cfg.F[3]),
            "bfcr": np.full((64, 1), np.float32(np.asarray(bfc).reshape(-1)[0])),
        }
        per_core.append(m)
    return per_core


# --------------------------------------------------------------------------
# Bass/Tile SPMD program
# --------------------------------------------------------------------------

def build_program(cfg: Cfg):
    import concourse.bacc as bacc
    import concourse.mybir as mybir
    import concourse.tile as tile

    dt = mybir.dt
    f32 = dt.float32
    bf16 = dt.bfloat16
    Alu = mybir.AluOpType
    Act = mybir.ActivationFunctionType

    NC, T, GRP, NBLK = cfg.NC, cfg.T, cfg.GRP, cfg.NBLK
    NPC, TOTAL, BLK = cfg.NPC, cfg.TOTAL, cfg.BLK
    G = cfg.G
    F0, F1, F2, F3 = cfg.F
    NGRP = T // GRP
    GR = GRP * P
    NCHUNK = cfg.NCHUNK
    CHTS, CBASE, CHT_MAX = cfg.CHTS, cfg.CBASE, cfg.CHT_MAX

    nq = int(os.environ.get("K_QUEUES", "4"))
    nc = bacc.Bacc("TRN2", target_bir_lowering=False, debug=False,
                   enable_asserts=False, num_devices=NC,
                   num_swdge_queues=nq)
    _qctr = [0]

    def next_q():
        q = _qctr[0] % nq
        _qctr[0] += 1
        return q

    def inp(name, shape, dtype=f32):
        return nc.dram_tensor(name, list(shape), dtype, kind="ExternalInput")

    xT = inp("xT", (F0, TOTAL), bf16)
    idx16 = inp("idx16", (P, NCHUNK * 8), dt.int16)
    dstf = inp("dstf", (P, NCHUNK), bf16)
    wf = inp("wf", (P, NCHUNK), bf16)
    dstf32 = inp("dstf32", (P, NCHUNK))
    wf32 = inp("wf32", (P, NCHUNK))
    batchf = inp("batchf", (P, T))
    iota_in = inp("iota", (P, P), bf16)
    ident64_in = inp("ident64", (64, 64))
    cinv_in = inp("cinv", (G, 1))
    W1_in = inp("W1dup", (F0, 128), bf16)
    W2_in = inp("W2", (F1, F2), bf16)
    W3_in = inp("W3dup", (F2, 128), bf16)
    Wfc_in = inp("Wfc", (F3, 1))
    b1_in = inp("b1c", (F1, 1))
    b2_in = inp("b2c", (F2, 1))
    b3_in = inp("b3r", (1, F3))
    bfc_in = inp("bfcr", (64, 1))
    out_t = nc.dram_tensor("out", [64, 1], f32, kind="ExternalOutput")
    debug = os.environ.get("K_DEBUG", "") == "1"
    if debug:
        dG1 = nc.dram_tensor("dG1", [256, 128], mybir.dt.bfloat16,
                             kind="ExternalOutput")
        dG2 = nc.dram_tensor("dG2", [256, 128], mybir.dt.bfloat16,
                             kind="ExternalOutput")
        dG3 = nc.dram_tensor("dG3", [256, 128], mybir.dt.bfloat16,
                             kind="ExternalOutput")
        dPool = nc.dram_tensor("dPool", [64, 64], f32,
                               kind="ExternalOutput")
        dPr = nc.dram_tensor("dPr", [64, 64], f32, kind="ExternalOutput")
        dST = nc.dram_tensor("dST", [64, 64], f32, kind="ExternalOutput")
        dFps = nc.dram_tensor("dFps", [64, 1], f32, kind="ExternalOutput")

    rg = [list(range(NC))]

    with tile.TileContext(nc) as tc:
        import contextlib
        ctx = contextlib.ExitStack()
        with ctx:
            dram = ctx.enter_context(tc.tile_pool(name="dram", bufs=1, space="DRAM"))
            pers = ctx.enter_context(tc.tile_pool(name="pers", bufs=1))
            sb2 = ctx.enter_context(tc.tile_pool(name="sb2", bufs=2))
            sb3 = ctx.enter_context(tc.tile_pool(name="sb3", bufs=3))
            spool = ctx.enter_context(tc.tile_pool(name="spool", bufs=4))
            gpool = ctx.enter_context(tc.tile_pool(name="gpool", bufs=6))
            xpool = ctx.enter_context(tc.tile_pool(name="xpool", bufs=2))
            gemm_ps = ctx.enter_context(tc.tile_pool(name="gemm_ps", bufs=2, space="PSUM"))
            agg_ps = ctx.enter_context(tc.tile_pool(name="agg_ps", bufs=3, space="PSUM"))
            tp_ps = ctx.enter_context(tc.tile_pool(name="tp_ps", bufs=1, space="PSUM"))

            # ---------- DRAM intermediates ----------
            G1 = dram.tile([TOTAL, 128], bf16, name="G1")
            G2_shard = dram.tile([NPC, 128], bf16, name="G2_shard")
            G3_shard = dram.tile([NPC, 128], bf16, name="G3_shard")
            pool_in = dram.tile([64, F3], f32, name="pool_in")

            # ---------- persistent SBUF ----------
            idx_sb = pers.tile([P, NCHUNK * 8], dt.int16, name="idx_sb")
            nc.sync.dma_start(idx_sb[:], idx16[:])
            sbuild = os.environ.get("K_SBUILD", "wide")
            if sbuild == "wide":
                dstf_sb = pers.tile([P, NCHUNK], bf16, name="dstf_sb")
                wf_sb = pers.tile([P, NCHUNK], bf16, name="wf_sb")
                nc.sync.dma_start(dstf_sb[:], dstf[:])
                nc.sync.dma_start(wf_sb[:], wf[:])
            else:
                dstf_sb = pers.tile([P, NCHUNK], f32, name="dstf_sb")
                wf_sb = pers.tile([P, NCHUNK], f32, name="wf_sb")
                nc.sync.dma_start(dstf_sb[:], dstf32[:])
                nc.sync.dma_start(wf_sb[:], wf32[:])
            iota_sb = pers.tile([P, P], bf16, name="iota_sb")
            ident64_sb = pers.tile([64, 64], f32, name="ident64_sb")
            cinv_sb = pers.tile([G, 1], f32, name="cinv_sb")
            batchf_sb = pers.tile([P, T], f32, name="batchf_sb")
            nc.sync.dma_start(iota_sb[:], iota_in[:])
            nc.sync.dma_start(ident64_sb[:], ident64_in[:])
            nc.sync.dma_start(cinv_sb[:], cinv_in[:])
            nc.sync.dma_start(batchf_sb[:], batchf[:])
            W1_sb = pers.tile([F0, 128], bf16, name="W1_sb")
            W2_sb = pers.tile([F1, F2], bf16, name="W2_sb")
            W3_sb = pers.tile([F2, 128], bf16, name="W3_sb")
            Wfc_sb = pers.tile([F3, 1], f32, name="Wfc_sb")
            nc.sync.dma_start(W1_sb[:], W1_in[:])
            nc.sync.dma_start(W2_sb[:], W2_in[:])
            nc.sync.dma_start(W3_sb[:], W3_in[:])
            nc.sync.dma_start(Wfc_sb[:], Wfc_in[:])
            b1_sb = pers.tile([F1, 1], f32, name="b1_sb")
            b2_sb = pers.tile([F2, 1], f32, name="b2_sb")
            b3_sb = pers.tile([1, F3], f32, name="b3_sb")
            bfc_sb = pers.tile([64, 1], f32, name="bfc_sb")
            nc.sync.dma_start(b1_sb[:], b1_in[:])
            nc.sync.dma_start(b2_sb[:], b2_in[:])
            nc.sync.dma_start(b3_sb[:], b3_in[:])
            nc.sync.dma_start(bfc_sb[:], bfc_in[:])
            pool_sb = pers.tile([64, F3], f32, name="pool_sb")
            nc.vector.memset(pool_sb[:], 0.0)

            # ================= gemm1: G1 = (x @ [W1|W1]) bf16, replicated ===
            def gemm1():
                for c in range(NC):
                    xpc = xpool.tile([F0, NPC], bf16, name="xpc", tag="xpc")
                    nc.sync.dma_start(xpc[:], xT[:, c * NPC:(c + 1) * NPC])
                    for g in range(NGRP):
                        xg = xpc[:, g * GR:(g + 1) * GR].rearrange(
                            "f (p j) -> f j p", j=GRP)
                        stage = sb3.tile([P, GRP * 128], bf16, name="g1st",
                                         tag="g1st")
                        for j in range(GRP):
                            ps = gemm_ps.tile([P, 128], f32, name="psg",
                                              tag="gps")
                            nc.tensor.matmul(ps[:], lhsT=xg[:, j, :],
                                             rhs=W1_sb[:], start=True,
                                             stop=True)
                            dst_sl = stage[:, j * 128:(j + 1) * 128]
                            if j % 2 == 0:
                                nc.scalar.copy(dst_sl, ps[:])
                            else:
                                nc.vector.tensor_copy(dst_sl, ps[:])
                        rows = G1[c * NPC + g * GR: c * NPC + (g + 1) * GR, :]
                        nc.sync.dma_start(
                            rows.rearrange("(p j) f -> p j f", j=GRP),
                            stage[:].rearrange("p (j f) -> p j f", j=GRP))

            # ================= aggregation layer ==========================
            def agg_layer(li):
                """li=0: consume G1, produce X2T->G2_shard (transposed agg)
                   li=1: consume G2_full, produce X3T->G3_shard (transposed)
                   li=2: consume G3_full, produce pooled partials (normal)."""
                g_src = (G1, G2_full, G3_full)[li]
                FI = (64, 128, 64)[li]       # real feature width of G rows
                transposed = li != 2
                for g in range(NGRP):
                    if li == 0:
                        stage = sb3.tile([P, GRP * F2], bf16, name="g2st",
                                         tag="g2st")
                    elif li == 1:
                        stage = sb3.tile([P, GRP * 128], bf16, name="g3st",
                                         tag="g3st")
                    else:
                        pp = tp_ps.tile([64, F3], f32, name="pp", tag="pp")
                    for j in range(GRP):
                        nch_t = sum(CHTS[(g, b, j)] for b in range(NBLK))
                        assert nch_t > 0
                        if transposed:
                            aps = agg_ps.tile([FI, P], f32, name="apsT",
                                              tag="aps")
                        else:
                            aps = agg_ps.tile([P, F3], f32, name="aps",
                                              tag="aps")
                        done = 0
                        for b in range(NBLK):
                            cht = CHTS[(g, b, j)]
                            if cht == 0:
                                continue
                            cb = CBASE[(g, b, j)]
                            gb = gpool.tile([P, CHT_MAX, 128], bf16,
                                            name="gb", tag="gb")
                            nc.gpsimd.dma_gather(
                                gb[:, :cht, :],
                                g_src[b * BLK:(b + 1) * BLK, :],
                                idx_sb[:, cb * 8:(cb + cht) * 8],
                                cht * P, cht * P, 128, single_packet=False,
                                queue_num=next_q())
                            S = spool.tile([P, CHT_MAX, 128], bf16,
                                           name="S", tag="S")
                            if sbuild == "wide":
                                # scale gathered rows by edge weights (norm)
                                wcol = wf_sb[:, cb:cb + cht]
                                gview = gb[:, :cht, :FI]
                                nc.vector.tensor_tensor(
                                    out=gview, in0=gview,
                                    in1=wcol[:, :, None].broadcast_to(
                                        [P, cht, FI]),
                                    op=Alu.mult)
                                # one-hot S for the bucket
                                nc.vector.tensor_tensor(
                                    out=S[:, :cht, :],
                                    in0=iota_sb[:, None, :].broadcast_to(
                                        [P, cht, 128]),
                                    in1=dstf_sb[:, cb:cb + cht][:, :, None]
                                    .broadcast_to([P, cht, 128]),
                                    op=Alu.is_equal)
                            else:
                                for k in range(cht):
                                    cc = cb + k
                                    nc.vector.tensor_scalar(
                                        S[:, k, :], iota_sb[:],
                                        dstf_sb[:, cc:cc + 1],
                                        wf_sb[:, cc:cc + 1],
                                        Alu.is_equal, op1=Alu.mult)
                            for k in range(cht):
                                done += 1
                                if transposed:
                                    nc.tensor.matmul(
                                        aps[:], lhsT=gb[:, k, :FI],
                                        rhs=S[:, k, :],
                                        start=(done == 1),
                                        stop=(done == nch_t))
                                else:
                                    nc.tensor.matmul(
                                        aps[:], lhsT=S[:, k, :],
                                        rhs=gb[:, k, :FI],
                                        start=(done == 1),
                                        stop=(done == nch_t))
                        # ---- epilogue ----
                        if li == 0:
                            # X2T = relu(aps + b1), then G2 = X2 @ W2
                            x2t = spool.tile([F1, P], bf16, name="x2t",
                                             tag="x2t")
                            nc.scalar.activation(x2t[:], aps[:], Act.Relu,
                                                 bias=b1_sb[:, 0:1])
                            ps2 = gemm_ps.tile([P, F2], f32, name="ps2",
                                               tag="gps")
                            nc.tensor.matmul(ps2[:], lhsT=x2t[:], rhs=W2_sb[:],
                                             start=True, stop=True)
                            nc.scalar.copy(stage[:, j * F2:(j + 1) * F2],
                                           ps2[:])
                        elif li == 1:
                            x3t = spool.tile([F2, P], bf16, name="x3t",
                                             tag="x3t")
                            nc.scalar.activation(x3t[:], aps[:], Act.Relu,
                                                 bias=b2_sb[:, 0:1])
                            ps3 = gemm_ps.tile([P, 128], f32, name="ps3",
                                               tag="gps")
                            nc.tensor.matmul(ps3[:], lhsT=x3t[:], rhs=W3_sb[:],
                                             start=True, stop=True)
                            nc.scalar.copy(stage[:, j * 128:(j + 1) * 128],
                                           ps3[:])
                        else:

                            x4 = spool.tile([P, F3], bf16, name="x4", tag="x4")
                            nc.scalar.activation(x4[:], aps[:], Act.Relu)
                            t = g * GRP + j
                            Bt = spool.tile([P, 64], bf16, name="Bt", tag="Bt")
                            nc.vector.tensor_scalar(
                                Bt[:], iota_sb[:, :64],
                                batchf_sb[:, t:t + 1], None, Alu.is_equal)
                            nc.tensor.matmul(pp[:], lhsT=Bt[:], rhs=x4[:],
                                             start=(j == 0), stop=(j == GRP - 1))
                    # ---- per-group flush ----
                    if li == 0:
                        rows = G2_shard[g * GR:(g + 1) * GR, :]
                        nc.sync.dma_start(
                            rows.rearrange("(p j) f -> p j f", j=GRP),
                            stage[:].rearrange("p (j f) -> p j f", j=GRP))
                    elif li == 1:
                        rows = G3_shard[g * GR:(g + 1) * GR, :]
                        nc.sync.dma_start(
                            rows.rearrange("(p j) f -> p j f", j=GRP),
                            stage[:].rearrange("p (j f) -> p j f", j=GRP))
                    else:
                        nc.vector.tensor_tensor(out=pool_sb[:],
                                                in0=pool_sb[:], in1=pp[:],
                                                op=Alu.add)

            # ================= execution =================
            cut = os.environ.get("K_CUT", "")

            def _cut(stage):
                return cut and cut == stage

            reps = int(os.environ.get("K_REPS", "1"))
            for _rep in range(reps):
                G2_full = dram.tile([TOTAL, 128], bf16,
                                    name=f"G2_full_{_rep}",
                                    addr_space="Shared")
                G3_full = dram.tile([TOTAL, 128], bf16,
                                    name=f"G3_full_{_rep}",
                                    addr_space="Shared")
                pool_out = dram.tile([64, F3], f32, name=f"pool_out_{_rep}",
                                     addr_space="Shared")
                if _rep > 0:
                    nc.vector.memset(pool_sb[:], 0.0)
                if _cut("prep"):
                    break
                gemm1()
                if _cut("gemm1"):
                    break
                agg_layer(0)
                if _cut("agg1pre"):
                    break
                nc.gpsimd.collective_compute(
                    "AllGather", Alu.bypass, ins=[G2_shard.opt()],
                    outs=[G2_full.opt()], replica_groups=rg)
                if _cut("agg1"):
                    break
                agg_layer(1)
                if _cut("agg2pre"):
                    break
                nc.gpsimd.collective_compute(
                    "AllGather", Alu.bypass, ins=[G3_shard.opt()],
                    outs=[G3_full.opt()], replica_groups=rg)
                if _cut("agg2"):
                    break
                agg_layer(2)
                if _cut("agg3"):
                    break

                # ---- pool + FC ----
                if debug:
                    for nm, srcb, dstb in (("dG1", G1, dG1), ("dG2", G2_shard, dG2),
                                           ("dG3", G3_shard, dG3)):
                        for q in range(2):
                            bt = sb2.tile([P, 128], mybir.dt.bfloat16,
                                          name=f"bt{nm}{q}", tag=f"bt{nm}")
                            nc.sync.dma_start(bt[:], srcb[q*128:(q+1)*128, :])
                            nc.sync.dma_start(dstb[q*128:(q+1)*128, :], bt[:])
                    nc.sync.dma_start(dPool[:], pool_sb[:])
                nc.sync.dma_start(pool_in[:], pool_sb[:])
                nc.gpsimd.collective_compute(
                    "AllReduce", Alu.add, ins=[pool_in.opt()],
                    outs=[pool_out.opt()], replica_groups=rg)
                pr = sb2.tile([64, F3], f32, name="pr", tag="pr")
                nc.sync.dma_start(pr[:], pool_out[:])
                tsp = tp_ps.tile([F3, 64], f32, name="tsp", tag="tp")
                nc.tensor.transpose(tsp[:], pr[:], ident64_sb[:])
                sT = sb2.tile([F3, 64], f32, name="sT", tag="sT")
                nc.vector.tensor_copy(sT[:], tsp[:])
                if debug:
                    nc.sync.dma_start(dPr[:], pr[:])
                    nc.sync.dma_start(dST[:], sT[:])
                fps = tp_ps.tile([64, 1], f32, name="fps", tag="tp")
                nc.tensor.matmul(fps[:], lhsT=sT[:], rhs=Wfc_sb[:], start=True,
                                 stop=True)
                res = sb2.tile([64, 1], f32, name="res", tag="res")
                nc.vector.tensor_scalar(res[:], fps[:], cinv_sb[:], bfc_sb[:],
                                        Alu.mult, op1=Alu.add)
                if debug:
                    fcp = sb2.tile([64, 1], f32, name="fcp", tag="fcp")
                    nc.vector.tensor_copy(fcp[:], fps[:])
                    nc.sync.dma_start(dFps[:], fcp[:])
                nc.sync.dma_start(out_t[:], res[:])

    nc.compile()
    return nc


# --------------------------------------------------------------------------
# Entry point
# --------------------------------------------------------------------------

_PROGRAM_CACHE = {}


def kernel(x, src, dst, edge_weight, batch, W1, b1, W2, b2, W3, b3, Wfc, bfc):
    from concourse.bass_utils import run_bass_kernel_spmd

    cfg = Cfg(**FULL_CFG)
    per_core = host_prep(x, src, dst, edge_weight, batch, W1, b1, W2, b2, W3,
                         b3, Wfc, bfc, cfg)
    key = (tuple(sorted(cfg.CHTS.items())), cfg.HAS_B3)
    if key not in _PROGRAM_CACHE:
        _PROGRAM_CACHE[key] = build_program(cfg)
    nc = _PROGRAM_CACHE[key]
    res = run_bass_kernel_spmd(nc, per_core, list(range(cfg.NC)))
    out = np.asarray(res.results[0]["out"], np.float32).reshape(cfg.G, 1)
    return out

